# revision 48
# baseline (speedup 1.0000x reference)
"""Trainium2 Bass kernel: row-softmax + embedding gather (batched lookup).

reference:
    probs = softmax(poi_freq_matrix, axis=1)        # [100000, 168] f32
    out   = probs[inputs_wekn]                      # [1024, 200, 168] f32

Strategy (8 NeuronCores, data-parallel over batch; each core owns 128
batch rows = 128 SBUF partitions x 200 seq positions).

Gather: the table is packed into [50000, 2x192] bf16 PAIR rows (768B =
3x256, a legal SWDGE stride).  int16 indices only reach 32768 rows, but
the gather ucode's address math is IVP_MULUSAN_2X32 (unsigned stride x
SIGNED idx), so with the in_ap base advanced to pair 32768 the signed
idx = wekn//2 - 32768 in [-32768, 17231] addresses all 50000 pairs.
This halves gather traffic vs the 4-row-quad layout (768B vs 1536B per
lookup) and needs ONE predicated sub-row select instead of three.
The ucode trims TRAILING negative idxs, so each chunk's list ends with
16 always-positive pad indices (one wrap column; gathers garbage pair
32768 into a scratch slot).

Emission: dma_gather descriptor emission runs on one Q7 CPU pair
selected by queue_num (~8ns/idx serial).  Gathers round-robin over all
4 SWDGE queues = 4 distinct CPU pairs; the Pool sequencer dispatches
ahead, so up to 4 emissions overlap (~2ns/idx effective).

Tail per chunk: one DVE copy_predicated picks the odd sub-row (uint32
pairs, 84/row); ACT exp -> bf16; DVE bf16 fold (168->84, 2x mode) +
tensor_reduce + reciprocal + one bulk tensor_tensor scale; bf16 HBM
store (f32 upcast on host).  GpSimd compute is avoided while DVE is
hot (they share an SBUF port).
"""

import sys

import numpy as np

sys.path.insert(0, "/opt/trn_rl_repo")

N_POI = 100000
N_BINS = 168
DP = 192  # padded row length in bf16 elems (384B)
NPAIR = N_POI // 2  # pair rows
IDX_BASE = 32768  # in_ap base offset; idx16 = pair_id - IDX_BASE
BATCH = 1024
SEQ = 200
N_CORES = 8
BPC = BATCH // N_CORES  # batch rows per core = 128 partitions

CHUNKS = (2, 5, 8, 11, 14, 20, 20, 20, 20, 20, 20, 20, 20)

_NC_CACHE = {}


def build(chunks=CHUNKS, nqueues=4, scratch=32768, tbufs=6, pbufs=6):
    """Build the per-core Bass program (SPMD: same NEFF on all cores)."""
    import concourse.bacc as bacc
    import concourse.tile as tile
    from concourse import bass, mybir

    seq = sum(chunks)
    mx = max(chunks)
    # idx columns (16-wrapped): 8 per position + 1 pad column per chunk
    wcols = 8 * seq + len(chunks)
    nc = bacc.Bacc(
        "TRN2",
        target_bir_lowering=False,
        debug=False,
        enable_asserts=False,
        num_devices=N_CORES,
        num_swdge_queues=nqueues,
        dynamic_dma_scratch_size=scratch,
        enable_partition_id=False,
    )
    ptab = nc.dram_tensor(
        "ptab", [NPAIR, 2 * DP], mybir.dt.bfloat16, kind="ExternalInput"
    ).ap()
    widx = nc.dram_tensor(
        "widx", [128, wcols], mybir.dt.int16, kind="ExternalInput"
    ).ap()
    msk = nc.dram_tensor(
        "msk", [BPC, seq], mybir.dt.uint8, kind="ExternalInput"
    ).ap()
    out = nc.dram_tensor(
        "out", [BPC, seq, N_BINS], mybir.dt.bfloat16, kind="ExternalOutput"
    ).ap()

    # issue the ~6us Q7 IRAM library load before the Tile preamble so it
    # overlaps the sem-clear/clock-load startup instead of serializing
    from concourse import library_config

    nc.gpsimd.load_library(library_config.mlp)

    with tile.TileContext(nc) as tc:
        with tc.tile_pool(name="const", bufs=1) as cpool, tc.tile_pool(
            name="quad", bufs=tbufs
        ) as tpool, tc.tile_pool(name="prob", bufs=pbufs) as ppool, tc.tile_pool(
            name="small", bufs=8
        ) as smpool, tc.tile_pool(name="foldp", bufs=4) as fpool:
            m0 = chunks[0]
            wt = cpool.tile([128, wcols], mybir.dt.int16)
            # chunk 0's idx slice loads first so its gather starts ASAP
            c0 = m0 * 8 + 1
            nc.sync.dma_start(out=wt[:, :c0], in_=widx[:, :c0])
            nc.sync.dma_start(out=wt[:, c0:], in_=widx[:, c0:])
            mt = cpool.tile([BPC, seq], mybir.dt.uint8)
            nc.sync.dma_start(out=mt[:], in_=msk[:])

            # --- two-deep software pipeline over the DVE stream ---
            # Per loop iteration k the DVE receives: select_k, then
            # fold/reduce/recip of chunk k-1 (whose ACT exp has had a full
            # chunk-time to land), then scale of chunk k-2 (whose ACT recb
            # likewise).  This removes the per-chunk DVE bubbles waiting on
            # the select->exp->fold and recip->recb->scale ACT hops.
            def stage_a(st, fast):
                """fold/reduce/recip (+recb on ACT); fast: direct reduce +
                1x scale + store (for the final chunks)."""
                m, off, Pf, P3 = st["m"], st["off"], st["Pf"], st["P3"]
                if not fast:
                    fold = fpool.tile(
                        [BPC, mx * 84], mybir.dt.bfloat16, tag="fold"
                    )
                    F3 = fold[:, : m * 84].rearrange("p (m d) -> p m d", m=m)
                    nc.vector.tensor_tensor(
                        out=F3,
                        in0=P3[:, :, 0:84],
                        in1=P3[:, :, 84:168],
                        op=mybir.AluOpType.add,
                    )
                    red_in = F3
                else:
                    red_in = P3
                sums = smpool.tile([BPC, mx], mybir.dt.float32, tag="sums")
                nc.vector.tensor_reduce(
                    out=sums[:, :m],
                    in_=red_in,
                    axis=mybir.AxisListType.X,
                    op=mybir.AluOpType.add,
                )
                rec = smpool.tile([BPC, mx], mybir.dt.float32, tag="rec")
                nc.vector.reciprocal(out=rec[:, :m], in_=sums[:, :m])
                if fast:
                    # pipeline end: 1x broadcast TT beats the extra
                    # DVE->ACT->DVE recb round trip in chain latency
                    nc.vector.tensor_tensor(
                        out=P3,
                        in0=P3,
                        in1=rec[:, :m].to_broadcast([BPC, m, N_BINS]),
                        op=mybir.AluOpType.mult,
                    )
                    nc.sync.dma_start(
                        out=out[:, off : off + m, :], in_=Pf[:, : m * N_BINS]
                    )
                    return None
                # ACT materializes the broadcast recip row in bf16 so the
                # DVE scale runs as one 2x-mode TT (a step-0 broadcast
                # last dim would lock tensor_tensor to 1x)
                recb = fpool.tile([BPC, mx * 84], mybir.dt.bfloat16, tag="recb")
                R3 = recb[:, : m * 84].rearrange("p (m d) -> p m d", m=m)
                nc.scalar.activation(
                    out=R3,
                    in_=rec[:, :m].to_broadcast([BPC, m, 84]),
                    func=mybir.ActivationFunctionType.Copy,
                )
                st["recb"] = recb
                return st

            def stage_b(st):
                """2x scale by the materialized recip row + store."""
                m, off, Pf, recb = st["m"], st["off"], st["Pf"], st["recb"]
                # one TT over [p, m, 2, 84]: in1 broadcasts on the middle
                # dim (step-0 there keeps the packed last dim, so 2x holds)
                P4v = Pf[:, : m * N_BINS].rearrange(
                    "p (m q d) -> p m q d", m=m, q=2
                )
                R4 = recb[:, : m * 84].rearrange(
                    "p (m q d) -> p m q d", m=m, q=1
                )
                nc.vector.tensor_tensor(
                    out=P4v,
                    in0=P4v,
                    in1=R4.to_broadcast([BPC, m, 2, 84]),
                    op=mybir.AluOpType.mult,
                )
                nc.sync.dma_start(
                    out=out[:, off : off + m, :], in_=Pf[:, : m * N_BINS]
                )

            off = 0  # position offset (output side)
            scol = 0  # idx column offset (8 per position + 1 pad col/chunk)
            pend_a = None  # chunk awaiting fold/reduce/recip (+recb)
            pend_b = None  # chunk awaiting scale+store
            for ci, m in enumerate(chunks):
                ms = m + 1  # gather-out slots (last holds the 16 pad rows)
                ni = BPC * m + 16  # idxs incl one positive 16-wrap pad col
                # full-size tiles (one tag each) sliced to this chunk
                Tf = tpool.tile([BPC, (mx + 1) * 2 * DP], mybir.dt.bfloat16, tag="T")
                T4 = Tf[:, : ms * 2 * DP].rearrange(
                    "p (m q d) -> p m q d", m=ms, q=2
                )
                nc.gpsimd.dma_gather(
                    out_ap=Tf[:, : ms * 2 * DP].rearrange(
                        "p (m d) -> p m d", m=ms
                    ),
                    in_ap=ptab[IDX_BASE:],
                    idxs_ap=wt[:, scol : scol + m * 8 + 1],
                    num_idxs=ni,
                    num_idxs_reg=ni,
                    elem_size=2 * DP,
                    elem_step=2 * DP,
                    single_packet=False,
                    queue_num=ci % nqueues,
                )
                # one select: odd sub-row over even, as uint32 pairs
                Tu = Tf[:, : ms * 2 * DP].bitcast(mybir.dt.uint32)
                U4 = Tu.rearrange("p (m q d) -> p m q d", m=ms, q=2)
                nc.vector.copy_predicated(
                    out=U4[:, :m, 0, : N_BINS // 2],
                    mask=mt[:, off : off + m].to_broadcast(
                        [BPC, m, N_BINS // 2]
                    ),
                    data=U4[:, :m, 1, : N_BINS // 2],
                )
                sel = T4[:, :m, 0, :N_BINS]
                Pf = ppool.tile([BPC, mx * N_BINS], mybir.dt.bfloat16, tag="P")
                P3 = Pf[:, : m * N_BINS].rearrange("p (m d) -> p m d", m=m)
                nc.scalar.activation(
                    out=P3, in_=sel, func=mybir.ActivationFunctionType.Exp
                )
                if pend_a is not None:
                    nxt = stage_a(pend_a, fast=False)
                    if pend_b is not None:
                        stage_b(pend_b)
                    pend_b = nxt
                pend_a = {"m": m, "off": off, "Pf": Pf, "P3": P3}
                off += m
                scol += m * 8 + 1
            # drain: the final chunk takes the fast path (no recb hop)
            nxt = stage_a(pend_a, fast=True)
            if pend_b is not None:
                stage_b(pend_b)
    nc.compile()
    return nc


def _prep_inputs(wekn, table, chunks=CHUNKS):
    """Host-side layout/index prep: bf16 cast, padded pair table, signed
    int16 pair ids (base-shifted), odd-row masks, per-core shards."""
    import ml_dtypes

    seq = sum(chunks)
    wcols = 8 * seq + len(chunks)
    tb = table.astype(ml_dtypes.bfloat16)
    pt = np.zeros((NPAIR, 2, DP), dtype=ml_dtypes.bfloat16)
    pt[:, :, :N_BINS] = tb.reshape(NPAIR, 2, N_BINS)
    pt = np.ascontiguousarray(pt.reshape(NPAIR, 2 * DP))
    in_maps = []
    for core in range(N_CORES):
        wc = wekn[core * BPC : (core + 1) * BPC]
        pair = (wc // 2 - IDX_BASE).astype(np.int16)
        sub = (wc % 2).astype(np.uint8)
        wi = np.empty((16, wcols), dtype=np.int16)
        off = 0
        scol = 0
        for m in chunks:
            ni = BPC * m + 16
            walk = np.empty(ni, dtype=np.int16)
            walk[: m * BPC] = pair[:, off : off + m].T.reshape(-1)
            walk[m * BPC :] = 0  # positive 16-pad (prevents trailing-trim)
            wi[:, scol : scol + m * 8 + 1] = walk.reshape(m * 8 + 1, 16).T
            off += m
            scol += m * 8 + 1
        in_maps.append(
            {
                "ptab": pt,
                "widx": np.tile(wi, (8, 1)),
                "msk": np.ascontiguousarray(sub),
            }
        )
    return in_maps


def _get_nc():
    if "nc" not in _NC_CACHE:
        _NC_CACHE["nc"] = build()
    return _NC_CACHE["nc"]


def kernel(**inputs) -> np.ndarray:
    wekn = np.asarray(inputs["inputs_wekn"]).astype(np.int64)
    table = np.ascontiguousarray(
        np.asarray(inputs["poi_freq_matrix"], dtype=np.float32)
    )
    assert wekn.shape == (BATCH, SEQ) and table.shape == (N_POI, N_BINS)

    from concourse.bass_utils import run_bass_kernel_spmd

    nc = _get_nc()
    in_maps = _prep_inputs(wekn, table)
    res = run_bass_kernel_spmd(nc, in_maps, core_ids=list(range(N_CORES)))
    return np.concatenate(
        [np.asarray(res.results[c]["out"]).astype(np.float32) for c in range(N_CORES)],
        axis=0,
    )


if __name__ == "__main__":
    rng = np.random.default_rng(0)
    inputs = {
        "venueid2coor": rng.random((N_POI, 2), dtype=np.float32),
        "inputs_wekn": rng.integers(0, N_POI, size=(BATCH, SEQ), dtype=np.int64),
        "poi_freq_matrix": rng.standard_normal((N_POI, N_BINS), dtype=np.float32),
    }
    out = kernel(**inputs)
    print(out.shape, out.dtype)


# revision 49
# speedup vs baseline: 1.0455x; 1.0455x over previous
"""Trainium2 Bass kernel: row-softmax + embedding gather (batched lookup).

reference:
    probs = softmax(poi_freq_matrix, axis=1)        # [100000, 168] f32
    out   = probs[inputs_wekn]                      # [1024, 200, 168] f32

Strategy (8 NeuronCores, data-parallel over batch; each core owns 128
batch rows = 128 SBUF partitions x 200 seq positions).

Gather: the table is packed into [50000, 2x192] bf16 PAIR rows (768B =
3x256, a legal SWDGE stride).  int16 indices only reach 32768 rows, but
the gather ucode's address math is IVP_MULUSAN_2X32 (unsigned stride x
SIGNED idx), so with the in_ap base advanced to pair 32768 the signed
idx = wekn//2 - 32768 in [-32768, 17231] addresses all 50000 pairs.
This halves gather traffic vs the 4-row-quad layout (768B vs 1536B per
lookup) and needs ONE predicated sub-row select instead of three.
The ucode trims TRAILING negative idxs, so each chunk's list ends with
16 always-positive pad indices (one wrap column; gathers garbage pair
32768 into a scratch slot).

Emission: dma_gather descriptor emission runs on one Q7 CPU pair
selected by queue_num (~8ns/idx serial).  Gathers round-robin over all
4 SWDGE queues = 4 distinct CPU pairs; the Pool sequencer dispatches
ahead, so up to 4 emissions overlap (~2ns/idx effective).

Tail per chunk: one DVE copy_predicated picks the odd sub-row (uint32
pairs, 84/row); ACT exp -> bf16; DVE bf16 fold (168->84, 2x mode) +
tensor_reduce + reciprocal + one bulk tensor_tensor scale; bf16 HBM
store (f32 upcast on host).  GpSimd compute is avoided while DVE is
hot (they share an SBUF port).
"""

import sys

import numpy as np

sys.path.insert(0, "/opt/trn_rl_repo")

N_POI = 100000
N_BINS = 168
DP = 192  # padded row length in bf16 elems (384B)
NPAIR = N_POI // 2  # pair rows
IDX_BASE = 32768  # in_ap base offset; idx16 = pair_id - IDX_BASE
BATCH = 1024
SEQ = 200
N_CORES = 8
BPC = BATCH // N_CORES  # batch rows per core = 128 partitions

CHUNKS = (2, 5, 8, 11, 13, 13, 13, 13, 13, 13, 13, 13, 13, 13, 13, 12, 9, 6, 4)

_NC_CACHE = {}


def build(chunks=CHUNKS, nqueues=4, scratch=32768, tbufs=8, pbufs=6):
    """Build the per-core Bass program (SPMD: same NEFF on all cores)."""
    import concourse.bacc as bacc
    import concourse.tile as tile
    from concourse import bass, mybir

    seq = sum(chunks)
    mx = max(chunks)
    # idx columns (16-wrapped): 8 per position + 1 pad column per chunk
    wcols = 8 * seq + len(chunks)
    nc = bacc.Bacc(
        "TRN2",
        target_bir_lowering=False,
        debug=False,
        enable_asserts=False,
        num_devices=N_CORES,
        num_swdge_queues=nqueues,
        dynamic_dma_scratch_size=scratch,
        enable_partition_id=False,
    )
    ptab = nc.dram_tensor(
        "ptab", [NPAIR, 2 * DP], mybir.dt.bfloat16, kind="ExternalInput"
    ).ap()
    widx = nc.dram_tensor(
        "widx", [128, wcols], mybir.dt.int16, kind="ExternalInput"
    ).ap()
    msk = nc.dram_tensor(
        "msk", [BPC, seq], mybir.dt.uint8, kind="ExternalInput"
    ).ap()
    out = nc.dram_tensor(
        "out", [BPC, seq, N_BINS], mybir.dt.bfloat16, kind="ExternalOutput"
    ).ap()

    # issue the ~6us Q7 IRAM library load before the Tile preamble so it
    # overlaps the sem-clear/clock-load startup instead of serializing
    from concourse import library_config

    nc.gpsimd.load_library(library_config.mlp)

    with tile.TileContext(nc) as tc:
        with tc.tile_pool(name="const", bufs=1) as cpool, tc.tile_pool(
            name="quad", bufs=tbufs
        ) as tpool, tc.tile_pool(name="prob", bufs=pbufs) as ppool, tc.tile_pool(
            name="small", bufs=8
        ) as smpool, tc.tile_pool(name="foldp", bufs=4) as fpool:
            m0 = chunks[0]
            wt = cpool.tile([128, wcols], mybir.dt.int16)
            # chunk 0's idx slice loads first so its gather starts ASAP
            c0 = m0 * 8 + 1
            nc.sync.dma_start(out=wt[:, :c0], in_=widx[:, :c0])
            nc.sync.dma_start(out=wt[:, c0:], in_=widx[:, c0:])
            mt = cpool.tile([BPC, seq], mybir.dt.uint8)
            nc.sync.dma_start(out=mt[:], in_=msk[:])

            # --- two-deep software pipeline over the DVE stream ---
            # Per loop iteration k the DVE receives: select_k, then
            # fold/reduce/recip of chunk k-1 (whose ACT exp has had a full
            # chunk-time to land), then scale of chunk k-2 (whose ACT recb
            # likewise).  This removes the per-chunk DVE bubbles waiting on
            # the select->exp->fold and recip->recb->scale ACT hops.
            def stage_a(st, fast):
                """fold/reduce/recip (+recb on ACT); fast: direct reduce +
                1x scale + store (for the final chunks)."""
                m, off, Pf, P3 = st["m"], st["off"], st["Pf"], st["P3"]
                if not fast:
                    fold = fpool.tile(
                        [BPC, mx * 84], mybir.dt.bfloat16, tag="fold"
                    )
                    F3 = fold[:, : m * 84].rearrange("p (m d) -> p m d", m=m)
                    nc.vector.tensor_tensor(
                        out=F3,
                        in0=P3[:, :, 0:84],
                        in1=P3[:, :, 84:168],
                        op=mybir.AluOpType.add,
                    )
                    red_in = F3
                else:
                    red_in = P3
                sums = smpool.tile([BPC, mx], mybir.dt.float32, tag="sums")
                nc.vector.tensor_reduce(
                    out=sums[:, :m],
                    in_=red_in,
                    axis=mybir.AxisListType.X,
                    op=mybir.AluOpType.add,
                )
                rec = smpool.tile([BPC, mx], mybir.dt.float32, tag="rec")
                nc.vector.reciprocal(out=rec[:, :m], in_=sums[:, :m])
                if fast:
                    # pipeline end: 1x broadcast TT beats the extra
                    # DVE->ACT->DVE recb round trip in chain latency
                    nc.vector.tensor_tensor(
                        out=P3,
                        in0=P3,
                        in1=rec[:, :m].to_broadcast([BPC, m, N_BINS]),
                        op=mybir.AluOpType.mult,
                    )
                    nc.sync.dma_start(
                        out=out[:, off : off + m, :], in_=Pf[:, : m * N_BINS]
                    )
                    return None
                # ACT materializes the broadcast recip row in bf16 so the
                # DVE scale runs as one 2x-mode TT (a step-0 broadcast
                # last dim would lock tensor_tensor to 1x)
                recb = fpool.tile([BPC, mx * 84], mybir.dt.bfloat16, tag="recb")
                R3 = recb[:, : m * 84].rearrange("p (m d) -> p m d", m=m)
                nc.scalar.activation(
                    out=R3,
                    in_=rec[:, :m].to_broadcast([BPC, m, 84]),
                    func=mybir.ActivationFunctionType.Copy,
                )
                st["recb"] = recb
                return st

            def stage_b(st):
                """2x scale by the materialized recip row + store."""
                m, off, Pf, recb = st["m"], st["off"], st["Pf"], st["recb"]
                # one TT over [p, m, 2, 84]: in1 broadcasts on the middle
                # dim (step-0 there keeps the packed last dim, so 2x holds)
                P4v = Pf[:, : m * N_BINS].rearrange(
                    "p (m q d) -> p m q d", m=m, q=2
                )
                R4 = recb[:, : m * 84].rearrange(
                    "p (m q d) -> p m q d", m=m, q=1
                )
                nc.vector.tensor_tensor(
                    out=P4v,
                    in0=P4v,
                    in1=R4.to_broadcast([BPC, m, 2, 84]),
                    op=mybir.AluOpType.mult,
                )
                nc.sync.dma_start(
                    out=out[:, off : off + m, :], in_=Pf[:, : m * N_BINS]
                )

            off = 0  # position offset (output side)
            scol = 0  # idx column offset (8 per position + 1 pad col/chunk)
            pend_a = None  # chunk awaiting fold/reduce/recip (+recb)
            pend_b = None  # chunk awaiting scale+store
            for ci, m in enumerate(chunks):
                ms = m + 1  # gather-out slots (last holds the 16 pad rows)
                ni = BPC * m + 16  # idxs incl one positive 16-wrap pad col
                # full-size tiles (one tag each) sliced to this chunk
                Tf = tpool.tile([BPC, (mx + 1) * 2 * DP], mybir.dt.bfloat16, tag="T")
                T4 = Tf[:, : ms * 2 * DP].rearrange(
                    "p (m q d) -> p m q d", m=ms, q=2
                )
                nc.gpsimd.dma_gather(
                    out_ap=Tf[:, : ms * 2 * DP].rearrange(
                        "p (m d) -> p m d", m=ms
                    ),
                    in_ap=ptab[IDX_BASE:],
                    idxs_ap=wt[:, scol : scol + m * 8 + 1],
                    num_idxs=ni,
                    num_idxs_reg=ni,
                    elem_size=2 * DP,
                    elem_step=2 * DP,
                    single_packet=False,
                    queue_num=ci % nqueues,
                )
                # one select: odd sub-row over even, as uint32 pairs
                Tu = Tf[:, : ms * 2 * DP].bitcast(mybir.dt.uint32)
                U4 = Tu.rearrange("p (m q d) -> p m q d", m=ms, q=2)
                nc.vector.copy_predicated(
                    out=U4[:, :m, 0, : N_BINS // 2],
                    mask=mt[:, off : off + m].to_broadcast(
                        [BPC, m, N_BINS // 2]
                    ),
                    data=U4[:, :m, 1, : N_BINS // 2],
                )
                sel = T4[:, :m, 0, :N_BINS]
                Pf = ppool.tile([BPC, mx * N_BINS], mybir.dt.bfloat16, tag="P")
                P3 = Pf[:, : m * N_BINS].rearrange("p (m d) -> p m d", m=m)
                nc.scalar.activation(
                    out=P3, in_=sel, func=mybir.ActivationFunctionType.Exp
                )
                if pend_a is not None:
                    nxt = stage_a(pend_a, fast=False)
                    if pend_b is not None:
                        stage_b(pend_b)
                    pend_b = nxt
                pend_a = {"m": m, "off": off, "Pf": Pf, "P3": P3}
                off += m
                scol += m * 8 + 1
            # drain: the final chunk takes the fast path (no recb hop)
            nxt = stage_a(pend_a, fast=True)
            if pend_b is not None:
                stage_b(pend_b)
    nc.compile()
    return nc


def _prep_inputs(wekn, table, chunks=CHUNKS):
    """Host-side layout/index prep: bf16 cast, padded pair table, signed
    int16 pair ids (base-shifted), odd-row masks, per-core shards."""
    import ml_dtypes

    seq = sum(chunks)
    wcols = 8 * seq + len(chunks)
    tb = table.astype(ml_dtypes.bfloat16)
    pt = np.zeros((NPAIR, 2, DP), dtype=ml_dtypes.bfloat16)
    pt[:, :, :N_BINS] = tb.reshape(NPAIR, 2, N_BINS)
    pt = np.ascontiguousarray(pt.reshape(NPAIR, 2 * DP))
    in_maps = []
    for core in range(N_CORES):
        wc = wekn[core * BPC : (core + 1) * BPC]
        pair = (wc // 2 - IDX_BASE).astype(np.int16)
        sub = (wc % 2).astype(np.uint8)
        wi = np.empty((16, wcols), dtype=np.int16)
        off = 0
        scol = 0
        for m in chunks:
            ni = BPC * m + 16
            walk = np.empty(ni, dtype=np.int16)
            walk[: m * BPC] = pair[:, off : off + m].T.reshape(-1)
            walk[m * BPC :] = 0  # positive 16-pad (prevents trailing-trim)
            wi[:, scol : scol + m * 8 + 1] = walk.reshape(m * 8 + 1, 16).T
            off += m
            scol += m * 8 + 1
        in_maps.append(
            {
                "ptab": pt,
                "widx": np.tile(wi, (8, 1)),
                "msk": np.ascontiguousarray(sub),
            }
        )
    return in_maps


def _get_nc():
    if "nc" not in _NC_CACHE:
        _NC_CACHE["nc"] = build()
    return _NC_CACHE["nc"]


def kernel(**inputs) -> np.ndarray:
    wekn = np.asarray(inputs["inputs_wekn"]).astype(np.int64)
    table = np.ascontiguousarray(
        np.asarray(inputs["poi_freq_matrix"], dtype=np.float32)
    )
    assert wekn.shape == (BATCH, SEQ) and table.shape == (N_POI, N_BINS)

    from concourse.bass_utils import run_bass_kernel_spmd

    nc = _get_nc()
    in_maps = _prep_inputs(wekn, table)
    res = run_bass_kernel_spmd(nc, in_maps, core_ids=list(range(N_CORES)))
    return np.concatenate(
        [np.asarray(res.results[c]["out"]).astype(np.float32) for c in range(N_CORES)],
        axis=0,
    )


if __name__ == "__main__":
    rng = np.random.default_rng(0)
    inputs = {
        "venueid2coor": rng.random((N_POI, 2), dtype=np.float32),
        "inputs_wekn": rng.integers(0, N_POI, size=(BATCH, SEQ), dtype=np.int64),
        "poi_freq_matrix": rng.standard_normal((N_POI, N_BINS), dtype=np.float32),
    }
    out = kernel(**inputs)
    print(out.shape, out.dtype)
